# revision 1
# baseline (speedup 1.0000x reference)
"""Trainium2 Bass kernel for the affine-transformer backsubstitution chain.

reference semantics (D=2048, L=8):
    Al = Au = A; bl = bu = b
    for s in 0..L-1 (history reversed):
        Al' = relu(Al) @ dAl + min(Al,0) @ dAu
        bl' = relu(Al) @ dbl + min(Al,0) @ dbu + bl
        Au' = relu(Au) @ dAu + min(Au,0) @ dAl
        bu' = relu(Au) @ dbu + min(Au,0) @ dbl + bu
    lower = relu(Al) @ lower_in + min(Al,0) @ upper_in + bl
    upper = relu(Au) @ upper_in + min(Au,0) @ lower_in + bu

Sharding: rows of Al/Au across 8 cores (256 rows each), history replicated.
Per core the state is kept TRANSPOSED ([2048 k-partitions, 256 m-free]) so the
history matrices act directly as matmul weights (out = lhsT.T @ rhs), and the
clamped copies are the state:
    mvA[k] = [ relu(AlT)[k] | min(AuT,0)[k] ]   (pairs with dAl weight tiles)
    mvB[k] = [ min(AlT,0)[k] | relu(AuT)[k] ]   (pairs with dAu weight tiles)
One [128,512] PSUM per output chunk then accumulates both chains at once:
    psum[:, :256] = sum_k dAl[k,n]·relu(AlT) + dAu[k,n]·min(AlT,0) = new AlT
    psum[:, 256:] = sum_k dAl[k,n]·min(AuT,0) + dAu[k,n]·relu(AuT) = new AuT
Compute dtype bf16 (fp32 PSUM accumulation); rel err vs fp32 ≈ 2.5e-3.

The bias chain (bl/bu) contributes only ~0.4% of the output magnitude, so its
per-step matvecs run in fp8-e4m3 with DoubleRow perf mode (2 k-chunks per
matmul) off fp8 shadow copies of the state written by the otherwise-idle
ScalarE. The final concretization against the input box stays bf16 (it
produces the full-magnitude output). Weight stripes are DMA'd as 2MB pairs
feeding two psum groups each. PE work: 4096 main + 128 bias + 32 final
matmuls x 216 ns ≈ 920 µs; measured ~938 µs/core on silicon (warm clocks;
occasional runs land ~1.13 ms when the chip drops to its 2.0 GHz P0 power
state under sustained load).
"""

import numpy as np
import ml_dtypes

L = 8
D = 2048
NCORES = 8
RPC = D // NCORES  # 256 rows per core
P = 128
KC = D // P  # 16 partition chunks
W = 2 * RPC  # 512: concatenated moving width

BF16 = ml_dtypes.bfloat16

_nc_cache = {}


def _build():
    from concourse import bacc
    import concourse.tile as tile
    import concourse.mybir as mybir

    dt = mybir.dt
    nc = bacc.Bacc()

    at0 = nc.dram_tensor("at0", [KC, P, RPC], dt.bfloat16, kind="ExternalInput")
    hist = nc.dram_tensor("hist", [L, KC // 2, 2, 2, P, D], dt.bfloat16, kind="ExternalInput")
    # hb8: fp8 bias-history vectors for DoubleRow matmuls. Per (step, family,
    # k-chunk-pair) a 32-byte block: byte 0 = chunk 2q's value, byte 16 =
    # chunk 2q+1's value (the 16-byte stride is a DoubleRow LDWEIGHTS
    # requirement).
    hb8 = nc.dram_tensor("hb8", [P, L * 2 * (KC // 2) * 32], dt.float8e4, kind="ExternalInput")
    fin = nc.dram_tensor("fin", [P, 2 * KC], dt.bfloat16, kind="ExternalInput")
    b2 = nc.dram_tensor("b2", [1, W], dt.float32, kind="ExternalInput")
    out = nc.dram_tensor("out", [1, W], dt.float32, kind="ExternalOutput")

    with tile.TileContext(nc) as tc:
        with (
            tc.tile_pool(name="state", bufs=1) as spool,
            tc.tile_pool(name="wts", bufs=3) as wpool,
            tc.tile_pool(name="consts", bufs=1) as cpool,
            tc.tile_pool(name="bias", bufs=1) as bpool,
            tc.tile_pool(name="psum", bufs=6, space="PSUM") as ppool,
            tc.tile_pool(name="psumb", bufs=2, space="PSUM") as pbpool,
        ):
            mvA = [spool.tile([P, KC * W], dt.bfloat16, tag=f"mvA{i}", name=f"mvA{i}") for i in range(2)]
            mvB = [spool.tile([P, KC * W], dt.bfloat16, tag=f"mvB{i}", name=f"mvB{i}") for i in range(2)]
            f8A = [spool.tile([P, KC * W], dt.float8e4, tag=f"f8A{i}", name=f"f8A{i}") for i in range(2)]
            f8B = [spool.tile([P, KC * W], dt.float8e4, tag=f"f8B{i}", name=f"f8B{i}") for i in range(2)]
            bst = [bpool.tile([1, W], dt.float32, tag=f"b{i}", name=f"b{i}") for i in range(2)]
            hb8t = cpool.tile([P, L * 2 * (KC // 2) * 32], dt.float8e4, tag="hb8t")
            fint = cpool.tile([P, 2 * KC], dt.bfloat16, tag="fint")

            # PE warmup: ~64 cheap matmuls on a zeroed tile run during the
            # initial DMA window so HAM un-throttles before the real stream.
            warm = cpool.tile([P, W], dt.bfloat16, tag="warm")
            nc.vector.memset(warm[:], 0.0)
            pw = ppool.tile([P, W], dt.float32, tag="ps", name="pw")
            for i in range(40):
                nc.tensor.matmul(pw[:, :P], warm[:, :P], warm[:, :P], start=True, stop=True)

            # Startup: interleave sub-DMAs of the first weight stripe with the
            # state-chunk loads on the Sync queue, in exact consumption order,
            # so the first psum group unblocks progressively. mvB is mvA with
            # the halves swapped — derive it on the (idle) DVE instead of
            # DMAing another 2MB.
            stripes = {}
            st00 = wpool.tile([P, 2, 2, D], dt.bfloat16, tag="stripe", name="stripe")
            stripes[(0, 0)] = st00
            h00 = hist[0, 0]
            for g in range(KC // 2):
                sl = slice(g * 2 * P, (g + 1) * 2 * P)
                # state chunks first: the DVE clamp chain hangs off them
                stgs = {}
                for i in (2 * g, 2 * g + 1):
                    stgs[i] = cpool.tile([P, RPC], dt.bfloat16, tag=f"stg{i%4}", name="stg")
                    nc.sync.dma_start(stgs[i][:], at0[i])
                nc.sync.dma_start(
                    st00[:, :, :, sl], h00[:, :, :, sl].rearrange("jh t p f -> p jh t f")
                )
                for i in (2 * g, 2 * g + 1):
                    # derive all four clamped slots on the (idle) DVE
                    o = i * W
                    stg = stgs[i]
                    # PL then NL first: they gate the chunk's first A/B matmuls
                    nc.vector.tensor_scalar_max(mvA[0][:, o : o + RPC], stg[:], 0.0)
                    nc.vector.tensor_scalar_min(mvB[0][:, o : o + RPC], stg[:], 0.0)
                    nc.vector.tensor_scalar_min(mvA[0][:, o + RPC : o + W], stg[:], 0.0)
                    nc.vector.tensor_scalar_max(mvB[0][:, o + RPC : o + W], stg[:], 0.0)
                    nc.scalar.copy(f8A[0][:, o : o + W], mvA[0][:, o : o + W])
                    nc.scalar.copy(f8B[0][:, o : o + W], mvB[0][:, o : o + W])
            st01 = wpool.tile([P, 2, 2, D], dt.bfloat16, tag="stripe", name="stripe")
            stripes[(0, 1)] = st01
            h01 = hist[0, 1]
            for g in range(4):
                sl = slice(g * 4 * P, (g + 1) * 4 * P)
                nc.sync.dma_start(
                    st01[:, :, :, sl], h01[:, :, :, sl].rearrange("jh t p f -> p jh t f")
                )
            nc.scalar.dma_start(bst[0][:], b2[:])
            nc.scalar.dma_start(hb8t[:], hb8[:])
            nc.scalar.dma_start(fint[:], fin[:])

            for s in range(L):
                cur, nxt = s % 2, (s + 1) % 2
                A, B = mvA[cur], mvB[cur]
                An, Bn = mvA[nxt], mvB[nxt]
                for jp in range(KC // 2):
                    if (s, jp) in stripes:
                        stripe = stripes.pop((s, jp))
                    else:
                        stripe = wpool.tile([P, 2, 2, D], dt.bfloat16, tag="stripe", name="stripe")
                        nc.sync.dma_start(
                            stripe[:], hist[s, jp].rearrange("jh t p f -> p jh t f")
                        )
                    for jh in range(2):
                        j = 2 * jp + jh
                        ps = ppool.tile([P, W], dt.float32, tag="ps", name="ps")
                        for i in range(KC):
                            nc.tensor.matmul(
                                ps[:],
                                stripe[:, jh, 0, i * P : (i + 1) * P],
                                A[:, i * W : (i + 1) * W],
                                start=(i == 0),
                                stop=False,
                            )
                            nc.tensor.matmul(
                                ps[:],
                                stripe[:, jh, 1, i * P : (i + 1) * P],
                                B[:, i * W : (i + 1) * W],
                                start=False,
                                stop=(i == KC - 1),
                            )
                        h = RPC
                        o = j * W
                        nc.vector.tensor_scalar_max(An[:, o : o + h], ps[:, :h], 0.0)
                        nc.vector.tensor_scalar_min(Bn[:, o : o + h], ps[:, :h], 0.0)
                        nc.vector.tensor_scalar_max(Bn[:, o + h : o + W], ps[:, h:], 0.0)
                        nc.vector.tensor_scalar_min(An[:, o + h : o + W], ps[:, h:], 0.0)
                        if s < L - 1:
                            # fp8 copies of the new state for the next step's
                            # DoubleRow bias matmuls (idle ScalarE).
                            nc.scalar.copy(f8A[nxt][:, o : o + W], An[:, o : o + W])
                            nc.scalar.copy(f8B[nxt][:, o : o + W], Bn[:, o : o + W])
                # bias chain: fp8 DoubleRow, 2 k-chunks per matmul. The bias is
                # ~0.4% of the output magnitude, so fp8 precision is ample.
                pb = pbpool.tile([1, W], dt.float32, tag="pb", name="pb")
                for q in range(KC // 2):
                    bl_blk = ((s * 2 + 0) * (KC // 2) + q) * 32
                    bu_blk = ((s * 2 + 1) * (KC // 2) + q) * 32
                    o2 = q * 2 * W
                    rhsA = f8A[cur][:, o2 : o2 + 2 * W].rearrange("p (o n) -> p o n", o=2)
                    rhsB = f8B[cur][:, o2 : o2 + 2 * W].rearrange("p (o n) -> p o n", o=2)
                    wl = hb8t[:, bl_blk : bl_blk + 32 : 16].rearrange("p (o m) -> p o m", m=1)
                    wu = hb8t[:, bu_blk : bu_blk + 32 : 16].rearrange("p (o m) -> p o m", m=1)
                    nc.tensor.matmul(
                        pb[:], wl, rhsA, start=(q == 0), stop=False,
                        perf_mode=mybir.MatmulPerfMode.DoubleRow,
                    )
                    nc.tensor.matmul(
                        pb[:], wu, rhsB, start=False, stop=(q == KC // 2 - 1),
                        perf_mode=mybir.MatmulPerfMode.DoubleRow,
                    )
                nc.vector.tensor_add(bst[nxt][:], pb[:], bst[cur][:])

            # final concretization against the input box
            Af, Bf = mvA[L % 2], mvB[L % 2]
            pf = pbpool.tile([1, W], dt.float32, tag="pb", name="pb")
            for i in range(KC):
                nc.tensor.matmul(
                    pf[:],
                    fint[:, i : i + 1],
                    Af[:, i * W : (i + 1) * W],
                    start=(i == 0),
                    stop=False,
                )
                nc.tensor.matmul(
                    pf[:],
                    fint[:, KC + i : KC + i + 1],
                    Bf[:, i * W : (i + 1) * W],
                    start=False,
                    stop=(i == KC - 1),
                )
            res = bpool.tile([1, W], dt.float32, tag="res")
            nc.vector.tensor_add(res[:], pf[:], bst[L % 2][:])
            nc.sync.dma_start(out[:], res[:])

    nc.finalize()
    return nc


def _get_nc():
    if "nc" not in _nc_cache:
        _nc_cache["nc"] = _build()
    return _nc_cache["nc"]


def _prep_inputs(A, b, hist_Al, hist_Au, hist_bl, hist_bu, lower_in, upper_in):
    A = np.asarray(A, dtype=np.float32)
    b = np.asarray(b, dtype=np.float32)
    hal = np.asarray(hist_Al, dtype=np.float32)[::-1]
    hau = np.asarray(hist_Au, dtype=np.float32)[::-1]
    hbl = np.asarray(hist_bl, dtype=np.float32)[::-1]
    hbu = np.asarray(hist_bu, dtype=np.float32)[::-1]
    lower_in = np.asarray(lower_in, dtype=np.float32)
    upper_in = np.asarray(upper_in, dtype=np.float32)

    # hist[s, j, t, p, i*P + n] = h_t[s, i*P + p, j*P + n], paired over j
    hist = np.empty([L, KC, 2, P, D], dtype=BF16)
    for t, h in enumerate((hal, hau)):
        hist[:, :, t] = (
            h.reshape(L, KC, P, KC, P).transpose(0, 3, 2, 1, 4).reshape(L, KC, P, D)
        )
    hist = hist.reshape(L, KC // 2, 2, 2, P, D)

    # hb8: per (step, family, chunk-pair) a 32-byte block with the paired
    # chunks' values at byte offsets 0 and 16 (DoubleRow weight layout).
    FP8 = ml_dtypes.float8_e4m3
    hb8 = np.zeros([P, L * 2 * (KC // 2), 32], dtype=FP8)
    for t, h in enumerate((hbl, hbu)):
        v = h.reshape(L, KC, P).astype(FP8)
        for s in range(L):
            for q in range(KC // 2):
                blk = (s * 2 + t) * (KC // 2) + q
                hb8[:, blk, 0] = v[s, 2 * q]
                hb8[:, blk, 16] = v[s, 2 * q + 1]
    hb8 = hb8.reshape(P, L * 2 * (KC // 2) * 32)

    # fin[p, t*KC + i]: t=0 lower_in, t=1 upper_in
    fin = (
        np.stack([lower_in.reshape(KC, P), upper_in.reshape(KC, P)], axis=0)
        .transpose(2, 0, 1)
        .reshape(P, 2 * KC)
        .astype(BF16)
    )

    in_maps = []
    for c in range(NCORES):
        At = np.ascontiguousarray(A[c * RPC : (c + 1) * RPC].T)  # [D, RPC]
        at0 = At.reshape(KC, P, RPC).astype(BF16)
        b_blk = b[c * RPC : (c + 1) * RPC]
        b2 = np.concatenate([b_blk, b_blk]).reshape(1, W).astype(np.float32)
        in_maps.append(
            {
                "at0": at0,
                "hist": hist,
                "hb8": hb8,
                "fin": fin,
                "b2": b2,
            }
        )
    return in_maps


def _run(in_maps, trace=False):
    from concourse.bass_utils import run_bass_kernel_spmd

    nc = _get_nc()
    return run_bass_kernel_spmd(
        nc, in_maps, core_ids=list(range(NCORES)), trace=trace
    )


def kernel(A, b, hist_Al, hist_Au, hist_bl, hist_bu, lower_in, upper_in):
    in_maps = _prep_inputs(
        A, b, hist_Al, hist_Au, hist_bl, hist_bu, lower_in, upper_in
    )
    res = _run(in_maps, trace=False)
    lower = np.concatenate([res.results[c]["out"][0, :RPC] for c in range(NCORES)])
    upper = np.concatenate([res.results[c]["out"][0, RPC:] for c in range(NCORES)])
    return lower.astype(np.float32), upper.astype(np.float32)

